# revision 1
# baseline (speedup 1.0000x reference)
"""Trainium2 Bass kernel for nn_CapsuleLayer (dynamic routing capsule layer).

Sharding: the 1152 input capsules (i) are split across 8 cores (144 each);
the full batch B=128 lives on SBUF partitions. Routing state (c, p) stays
local to each core's i-shard; the per-iteration s partial sums are combined
with 3 small AllReduces ([128,160] f32). u_hat is never materialized — both
big contractions are rewritten through W:
  s[b,j,d]       = sum_{i,k} (exp(c)/sigma)[b,j,i] x[b,i,k] W[j,i,d,k]  (PE)
  c_delta[b,j,i] = sum_k x[b,i,k] m[b,j,i,k],   m = sum_d v[b,j,d] W[j,i,d,k]
(m via PE d-contraction in float32r; the x-multiply fuses with PSUM evac.)
"""

import sys

if "/opt/trn_rl_repo" not in sys.path:
    sys.path.insert(0, "/opt/trn_rl_repo")

import contextlib

import numpy as np

import concourse.bass as bass  # noqa: F401
import concourse.tile as tile
from concourse import bacc, mybir
from concourse.bass_utils import run_bass_kernel_spmd
from concourse.masks import make_identity

f32 = mybir.dt.float32
f32r = mybir.dt.float32r
bf16 = mybir.dt.bfloat16
AL = mybir.AluOpType
AF = mybir.ActivationFunctionType

B = 128          # batch (on partitions)
NJ = 10          # output capsules
DO = 16          # output capsule dim
DI = 8           # input capsule dim
NI = 1152        # input capsules (global)
ROUTINGS = 3
EPS = 1e-7


def _stt(eng, out, in0, in1, op1, scalar=None, op0=None):
    if scalar is not None and scalar != 1.0 or op0 is not None and op0 != AL.mult:
        eng.scalar_tensor_tensor(out=out, in0=in0, scalar=scalar or 1.0,
                                 in1=in1, op0=op0 or AL.mult, op1=op1)
    else:
        eng.tensor_tensor(out=out, in0=in0, in1=in1, op=op1)


def build_kernel(n_cores=8, debug=False, repeat=1, single=False, ablate=()):
    ni_l = NI // n_cores
    chunks = []
    o = 0
    while o < ni_l:
        chunks.append((o, min(128, ni_l - o)))
        o += 128

    nc = bacc.Bacc("TRN2", target_bir_lowering=False, debug=False,
                   num_devices=1 if single else n_cores)
    x_d = nc.dram_tensor("x", [B, ni_l, DI], f32, kind="ExternalInput")
    w_d = nc.dram_tensor("w", [NJ, ni_l, DO, DI], f32, kind="ExternalInput")
    out_d = nc.dram_tensor("out", [B, NJ, DO], f32, kind="ExternalOutput")
    dbg = {}
    if debug:
        dbg["c"] = nc.dram_tensor("dbg_c", [B, NJ, ni_l], f32, kind="ExternalOutput")
        dbg["p"] = nc.dram_tensor("dbg_p", [B, NJ, ni_l], f32, kind="ExternalOutput")
        dbg["s0"] = nc.dram_tensor("dbg_s0", [B, DO, NJ], f32, kind="ExternalOutput")

    with tile.TileContext(nc) as tc:
        for _rep in range(repeat):
            _body(nc, tc, x_d, w_d, out_d, dbg if _rep == repeat - 1 else {},
                  ni_l, chunks, n_cores, single, ablate)
    nc.compile()
    return nc


def _body(nc, tc, x_d, w_d, out_d, dbg, ni_l, chunks, n_cores, single=False, ablate=()):
    ctx = contextlib.ExitStack()
    with ctx:
        sb = ctx.enter_context(tc.tile_pool(name="sb", bufs=1))
        sc = ctx.enter_context(tc.tile_pool(name="scratch", bufs=3))
        ps = ctx.enter_context(tc.tile_pool(name="ps", bufs=2, space="PSUM"))
        psm_pool = ps
        ps_acc = ctx.enter_context(tc.tile_pool(name="ps_acc", bufs=2, space="PSUM"))
        dram = ctx.enter_context(tc.tile_pool(name="dram", bufs=1, space="DRAM"))

        n_sl = (ni_l * DI) // 384
        # ---------------- Phase 0: loads + casts ----------------
        x_f = sc.tile([B, ni_l * DI], f32, tag="xload")
        nc.sync.dma_start(out=x_f, in_=x_d.ap().rearrange("b i k -> b (i k)"))
        x_bf = sb.tile([B, ni_l, DI], bf16)
        nc.vector.tensor_copy(out=x_bf.rearrange("b i k -> b (i k)"), in_=x_f)

        # natural W: [(i)ch, j, d, k] f32 -> bf16
        w_bf = []
        for c0, cn in chunks:
            wf = sc.tile([cn, NJ, DO, DI], f32, tag="wload")
            for _jh in range(2):
                _js = slice(_jh * NJ // 2, (_jh + 1) * NJ // 2)
                nc.sync.dma_start(
                    out=wf[:, _js, :, :],
                    in_=w_d.ap()[_js, c0:c0 + cn, :, :].rearrange("j i d k -> i j d k"))
            wb = sb.tile([cn, NJ, DO, DI], bf16, tag=f"wbf{c0}")
            nc.vector.tensor_copy(out=wb, in_=wf)
            w_bf.append(wb)

        # W_D: [(d)16p, j, i, k] f32 (for m-matmuls, used as f32r)
        w_dT = sb.tile([DO, NJ, ni_l, DI], f32r)

        ident = sb.tile([128, 128], bf16)
        make_identity(nc, ident)
        ident_f = sb.tile([128, 128], f32)
        make_identity(nc, ident_f)

        # x_P: [(i)ch, k, b] bf16 via PE transposes of k-slices
        x_P = [sb.tile([cn, DI, B], bf16, tag=f"xP{c0}", name=f"xP{c0}") for c0, cn in chunks]
        for ci, (c0, cn) in enumerate(chunks):
            for k in range(DI):
                pt = ps.tile([128, B], bf16, tag="tp", name="pt", padded_shape=[B, n_sl * 512])
                nc.tensor.transpose(pt[:cn, :], x_bf[:, c0:c0 + cn, k], ident)
                nc.vector.tensor_copy(out=x_P[ci][:, k, :], in_=pt[:cn, :])

        # ---------------- r0: s0 = (1/NJ) * sum_ik x W ----------------
        ps_s0 = ps_acc.tile([B, NJ, DO], f32, tag="smm")
        nmm = len(chunks) * DI
        imm = 0
        for ci, (c0, cn) in enumerate(chunks):
            for k in range(DI):
                nc.tensor.matmul(
                    ps_s0.rearrange("b j d -> b (j d)"),
                    lhsT=x_P[ci][:, k, :],
                    rhs=w_bf[ci][:, :, :, k].rearrange("i j d -> i (j d)"),
                    start=(imm == 0), stop=(imm == nmm - 1),
                )
                imm += 1
        s_part = sb.tile([B, DO, NJ], f32)
        nc.scalar.mul(out=s_part, in_=ps_s0.rearrange("b j d -> b d j"),
                      mul=1.0 / NJ)
        if dbg:
            nc.sync.dma_start(out=dbg["s0"].ap(), in_=s_part)

        # persistent state tiles
        c_t = sb.tile([B, NJ, ni_l], f32)        # routing logits (j, i)
        s_full = sb.tile([B, DO, NJ], f32)       # all-reduced s
        v_f = sb.tile([B, DO, NJ], f32)          # squashed v (d, j)
        v_T = sb.tile([DO, NJ, B], f32r)         # v transposed [(d), j, b]
        e_bf = sb.tile([B, NJ, ni_l], bf16)      # exp(c)
        ssum = sb.tile([B, ni_l], f32)           # sum_j exp(c)
        rin = sb.tile([B, ni_l], f32)            # 1/ssum
        rin_bf = sb.tile([B, ni_l], bf16)
        rin_T = [sb.tile([cn, B], bf16, tag=f"rinT{c0}", name=f"rinT{c0}") for c0, cn in chunks]
        xs_P = [sb.tile([cn, DI, B], bf16, tag=f"xsP{c0}", name=f"xsP{c0}") for c0, cn in chunks]
        t_all = sb.tile([B, NJ, ni_l, DI], bf16)  # m * x scratch (all j)
        sq = sb.tile([B, NJ], f32)
        fac = sb.tile([B, NJ], f32)
        eps_t = sb.tile([B, 1], f32)
        nc.vector.memset(eps_t, EPS)

        ar_in = dram.tile([B, DO * NJ], f32)
        ar_out = dram.tile([B, DO * NJ], f32)

        def transp(out_t, in_ap, is_bf):
            # out_t: [cn, B] SBUF tile; in_ap: [B, cn] AP (contiguous for DMA-T)
            cn = in_ap.shape[1]
            if cn % 128 == 0 and is_bf and in_ap.ap[-1][0] == 1:
                nc.sync.dma_start(out=out_t, in_=in_ap, transpose=True)
            else:
                pt = ps.tile([128, B], bf16 if is_bf else f32, tag="tp", name="pt", padded_shape=[B, n_sl * 512])
                nc.tensor.transpose(pt[:cn, :], in_ap, ident if is_bf else ident_f)
                nc.vector.tensor_copy(out=out_t, in_=pt[:cn, :])

        def allreduce_s():
            nc.sync.dma_start(out=ar_in, in_=s_part.rearrange("b d j -> b (d j)"))
            if single:
                nc.sync.dma_start(out=ar_out, in_=ar_in)
            else:
                nc.gpsimd.collective_compute(
                    "AllReduce", AL.add,
                    ins=[ar_in.opt()], outs=[ar_out.opt()],
                    replica_groups=[list(range(n_cores))],
                )
            nc.sync.dma_start(out=s_full.rearrange("b d j -> b (d j)"), in_=ar_out)

        def squash(last):
            # sq = sum_d s^2 ; v = s * sq/(1+sq)/sqrt(sq+eps)
            t = sc.tile([B, DO, NJ], f32, tag="sqt")
            nc.vector.tensor_mul(out=t, in0=s_full, in1=s_full)
            nc.vector.tensor_reduce(
                out=sq, in_=t.rearrange("b d j -> b j d"),
                axis=mybir.AxisListType.X, op=AL.add)
            srt = sc.tile([B, NJ], f32, tag="srt")
            nc.scalar.activation(out=srt, in_=sq, func=AF.Sqrt, bias=eps_t)
            den = sc.tile([B, NJ], f32, tag="den")
            nc.vector.scalar_tensor_tensor(out=den, in0=sq, scalar=1.0,
                                           in1=srt, op0=AL.add, op1=AL.mult)
            nc.vector.reciprocal(out=den, in_=den)
            nc.vector.tensor_mul(out=fac, in0=sq, in1=den)
            if last:
                # v in (j, d) order, contiguous for the output DMA
                v_out = sb.tile([B, NJ, DO], f32)
                nc.vector.tensor_mul(
                    out=v_out, in0=s_full.rearrange("b d j -> b j d"),
                    in1=fac.unsqueeze(2).broadcast_to([B, NJ, DO]))
                return v_out
            nc.vector.tensor_mul(
                out=v_f, in0=s_full,
                in1=fac.unsqueeze(1).broadcast_to([B, DO, NJ]))
            # v_T[(d), j, b] via per-j PE transposes (f32)
            for j in range(NJ):
                ptv = ps.tile([128, B], f32, tag="tp", name="ptv", padded_shape=[B, n_sl * 512])
                nc.tensor.transpose(ptv[:DO, :], v_f[:, :, j], ident_f)
                nc.vector.tensor_copy(out=v_T[:, j, :], in_=ptv[:DO, :])
            return None

        w_dik = w_dT.rearrange("d j i k -> d j (i k)")
        x_ik = x_bf.rearrange("b i k -> b (i k)")

        def c_update(first):
            # m_j = sum_d v[b,j,d] W[j,:,d,:] ; c += sum_k x*m  (batched over j)
            t_flat = t_all.rearrange("b j i k -> b (j i k)")
            for j in range(NJ):
                pm3 = psm_pool.tile([B, n_sl, 512], f32, tag="tp", name="pm3")
                for sl in range(n_sl):
                    nc.tensor.matmul(
                        pm3[:, sl, 0:384],
                        lhsT=v_T[:, j, :],
                        rhs=w_dik[:, j, 384 * sl:384 * (sl + 1)],
                        start=True, stop=True,
                    )
                if j % 3 == 2:
                    _stt(nc.vector,
                         out=t_flat[:, j * ni_l * DI:(j + 1) * ni_l * DI].rearrange(
                             "b (s e) -> b s e", s=n_sl),
                         in0=pm3[:, :, 0:384],
                         in1=x_ik.rearrange("b (s e) -> b s e", s=n_sl),
                         op1=AL.mult)
                else:
                    m_bf = sc.tile([B, n_sl, 384], bf16, tag="m_bf", name="m_bf")
                    nc.scalar.copy(out=m_bf, in_=pm3[:, :, 0:384])
                    _stt(nc.vector,
                         out=t_flat[:, j * ni_l * DI:(j + 1) * ni_l * DI],
                         in0=m_bf.rearrange("b s e -> b (s e)"),
                         in1=x_ik, op1=AL.mult)
            if "c_mul" in ablate:
                nc.vector.memset(c_t, 0.0)
                return
            # k-tree per j-half: 8 -> 4 -> 2 -> (+c)
            for jh in range(2):
                jsl = slice(jh * NJ // 2, (jh + 1) * NJ // 2)
                th = t_all[:, jsl, :, :]
                _stt(nc.vector, out=th[:, :, :, 0:4], in0=th[:, :, :, 0:4],
                     in1=th[:, :, :, 4:8], op1=AL.add)
                _stt(nc.vector, out=th[:, :, :, 0:2], in0=th[:, :, :, 0:2],
                     in1=th[:, :, :, 2:4], op1=AL.add)
                cv = c_t[:, jsl, :]
                if first:
                    _stt(nc.vector, out=cv, in0=th[:, :, :, 0],
                         in1=th[:, :, :, 1], op1=AL.add)
                else:
                    _stt(nc.vector, out=cv, in0=cv, in1=th[:, :, :, 0], op1=AL.add)
                    _stt(nc.vector, out=cv, in0=cv, in1=th[:, :, :, 1], op1=AL.add)

        def softmax_and_s():
            # e = exp(c); sigma = sum_j e; fold 1/sigma into x'
            for jh in range(2):
                jsl = slice(jh * NJ // 2, (jh + 1) * NJ // 2)
                nc.scalar.activation(out=e_bf[:, jsl, :], in_=c_t[:, jsl, :],
                                     func=AF.Exp)
            nc.vector.tensor_reduce(out=ssum,
                                    in_=e_bf.rearrange("b j i -> b i j"),
                                    axis=mybir.AxisListType.X, op=AL.add)
            nc.vector.reciprocal(out=rin, in_=ssum)
            nc.vector.tensor_copy(out=rin_bf, in_=rin)
            for ci, (c0, cn) in enumerate(chunks):
                transp(rin_T[ci], rin_bf[:, c0:c0 + cn], True)
                _stt(nc.vector, out=xs_P[ci], in0=x_P[ci],
                     in1=rin_T[ci].unsqueeze(1).broadcast_to([cn, DI, B]),
                     op1=AL.mult)
            if "s_tp" in ablate:
                return
            ps_s = ps_acc.tile([B, NJ, DO], f32, tag="smm", name="ps_s")
            for j in range(NJ):
                for ci, (c0, cn) in enumerate(chunks):
                    eT = sc.tile([cn, B], bf16, tag=f"eT{ci}", name="eT")
                    transp(eT, e_bf[:, j, c0:c0 + cn], True)
                    y = sc.tile([cn, DI, B], bf16, tag=f"y{ci}", name="y")
                    y_eng = nc.gpsimd if (ci == 1 and j % 2 == 1) else nc.vector
                    _stt(y_eng, out=y, in0=xs_P[ci],
                         in1=eT.unsqueeze(1).broadcast_to([cn, DI, B]),
                         op1=AL.mult)
                    if "s_mm" in ablate:
                        continue
                    for k in range(DI):
                        nc.tensor.matmul(
                            ps_s[:, j, :],
                            lhsT=y[:, k, :],
                            rhs=w_bf[ci][:, j, :, k],
                            start=(ci == 0 and k == 0),
                            stop=(ci == len(chunks) - 1 and k == DI - 1),
                        )
            if "s_mm" not in ablate:
                nc.vector.tensor_copy(out=s_part,
                                      in_=ps_s.rearrange("b j d -> b d j"))

        # ---------------- routing ----------------
        if {"s_tp", "s_y", "s_mm"} & set(ablate):
            nc.vector.memset(s_part, 0.0)
        allreduce_s()          # r0 s
        squash(last=False)     # r0 v
        for _j in range(NJ):
            nc.sync.dma_start(
                out=w_dT[:, _j, :, :],
                in_=w_d.ap()[_j].rearrange("i d k -> d i k").bitcast(f32r))
        if "cupd" not in ablate:
            c_update(first=True)   # c1
        v_out = None
        for r in range(1, ROUTINGS):
            last = (r == ROUTINGS - 1)
            if "smax" not in ablate:
                softmax_and_s()
            allreduce_s()
            v_out = squash(last=last)
            if not last and "cupd" not in ablate:
                c_update(first=False)
        if dbg:
            nc.sync.dma_start(out=dbg["c"].ap(), in_=c_t)
            p_f = sb.tile([B, NJ, ni_l], f32)
            nc.vector.tensor_mul(
                out=p_f, in0=e_bf,
                in1=rin.unsqueeze(1).broadcast_to([B, NJ, ni_l]))
            nc.sync.dma_start(out=dbg["p"].ap(), in_=p_f)

        nc.sync.dma_start(out=out_d.ap(), in_=v_out)


_NC_CACHE = {}


def kernel(inputs: np.ndarray, W: np.ndarray) -> np.ndarray:
    n_cores = 8
    ni_l = NI // n_cores
    if "nc" not in _NC_CACHE:
        _NC_CACHE["nc"] = build_kernel(n_cores=n_cores, debug=False)
    nc = _NC_CACHE["nc"]
    in_maps = []
    for r in range(n_cores):
        sl = slice(ni_l * r, ni_l * (r + 1))
        in_maps.append({
            "x": np.ascontiguousarray(inputs[:, sl, :], dtype=np.float32),
            "w": np.ascontiguousarray(W[:, sl, :, :], dtype=np.float32),
        })
    res = run_bass_kernel_spmd(nc, in_maps, core_ids=list(range(n_cores)))
    return res.results[0]["out"]

